# revision 52
# baseline (speedup 1.0000x reference)
"""KroneckerLSTM trn2 kernel.

Computes, for 8 gate-klins (L @ t @ R + b, t in {x,h}):
    i = sigmoid(klin_ii(x) + klin_hi(h)); f = sigmoid(...); g = tanh(...); o = sigmoid(...)
    c_new = f*c + i*g ; h_new = o*tanh(c_new)
Returns (h_new, c_new), each [1024,1024] f32.

Sharding: output rows split across 8 cores (128 rows each) -> zero collectives.
All matmul operands are bf16 (PSUM accumulation is fp32), halving HBM traffic;
the per-core stream is ~24 MB against a ~358 GB/s per-NC HBM limit, so the
schedule is built to keep the DMA queue saturated end-to-end:

  mm1-x (PE-dense, x streamed)                      | DMA: x, ltx, h, lth
  mm1-h {i,f} (h resident)  -> mm2 i -> mm2 f       | DMA: R_i, R_f streams
  mm1-h {g,o}               -> mm2 g -> mm2 o       | DMA: R_g, R_o streams
  c_new chain overlaps the o matmuls; epilogue per 512-col half.

Interleaving half of mm1 between the mm2 pair blocks keeps Tensor-engine duty
high through the DMA-paced mm2 stream, which keeps the PE HAM clock at
2.4 GHz (an idle-ish PE re-throttles to 1.2 GHz and then gates DMA buffer
recycling below the HBM rate).

mm1 computes A^T directly (lhsT = t tiles (natural), rhs = host-pretransposed
L^T column-slices stacked across gates), so mm1's PSUM output is the lhsT for
mm2 (rhs = R in natural layout).  The x- and h-klins of each gate pair
accumulate into the same PSUM bank; bias is added in-place in PSUM.
"""

import sys

import numpy as np

if "/opt/trn_rl_repo" not in sys.path:
    sys.path.insert(0, "/opt/trn_rl_repo")

N = 1024
M = 1024
P = 128
NC = 8
KT = N // P  # 8 k-tiles of 128
# gate pairs in order i, f, g, o: (x-gate, h-gate, activation)
PAIRS = [("ii", "hi", "Sigmoid"), ("if", "hf", "Sigmoid"),
         ("ig", "hg", "Tanh"), ("io", "ho", "Sigmoid")]

_cache = {}


def _build_program():
    import concourse.bass as bass
    import concourse.mybir as mybir
    import concourse.tile as tile
    from concourse import bacc
    from concourse.bass import ts

    FP = mybir.dt.float32
    FIN = mybir.dt.bfloat16  # matmul operand dtype (halves HBM traffic)
    AF = mybir.ActivationFunctionType

    nc = bacc.Bacc("TRN2", target_bir_lowering=False, debug=False,
                   enable_asserts=False, num_devices=NC)

    # inputs are host-repacked so one DMA moves several 128-row k-tiles
    # side-by-side in the free dim (fewer, larger transfers on the DMA queue):
    #   x/h:   [512, 2048]   — k-tiles (2kc, 2kc+1) side by side
    #   ltx:   [512, 1024]   — k-tiles (2kc, 2kc+1) of [128, 512]
    #   lth:   [256, 2048]   — per group: 4 k-tiles of [128, 256]
    #   R:     [256, 4096]   — k-tiles (4kc..4kc+3) of [128, 1024]
    #   bsum:  [128, 4096]   — 4 gate-pair biases side by side
    x_d = nc.dram_tensor("x", [N // 2, 2 * M], FIN, kind="ExternalInput").ap()
    h_d = nc.dram_tensor("h", [N // 2, 2 * M], FIN, kind="ExternalInput").ap()
    ltx_d = nc.dram_tensor("ltx", [N // 2, 8 * P], FIN, kind="ExternalInput").ap()
    lth_d = nc.dram_tensor("lth", [N // 4, 16 * P], FIN, kind="ExternalInput").ap()
    rx_d = [nc.dram_tensor(f"rx{p}", [M // 4, 4 * M], FIN, kind="ExternalInput").ap()
            for p in range(4)]
    rh_d = [nc.dram_tensor(f"rh{p}", [M // 4, 4 * M], FIN, kind="ExternalInput").ap()
            for p in range(4)]
    bs_d = nc.dram_tensor("bsum", [P, 4 * M], FIN, kind="ExternalInput").ap()
    c_d = nc.dram_tensor("cprev", [P, M], FIN, kind="ExternalInput").ap()
    eye_d = nc.dram_tensor("eye", [P, P], FIN, kind="ExternalInput").ap()
    hn_d = nc.dram_tensor("h_new", [P, M], FIN, kind="ExternalOutput").ap()
    cn_d = nc.dram_tensor("c_new", [P, M], FIN, kind="ExternalOutput").ap()

    with tile.TileContext(nc) as tc:
        from contextlib import ExitStack
        with ExitStack() as ctx:
            tin = ctx.enter_context(tc.tile_pool(name="tin", bufs=5))
            hrp = ctx.enter_context(tc.tile_pool(name="hres", bufs=1))
            ltp = ctx.enter_context(tc.tile_pool(name="lt", bufs=4))
            lhp = ctx.enter_context(tc.tile_pool(name="lth", bufs=1))
            atxp = ctx.enter_context(tc.tile_pool(name="atx", bufs=1))
            athp = ctx.enter_context(tc.tile_pool(name="ath", bufs=1))
            rp = ctx.enter_context(tc.tile_pool(name="rstream", bufs=8))
            psp = ctx.enter_context(tc.tile_pool(name="ps", bufs=8, space="PSUM"))
            bsp = ctx.enter_context(tc.tile_pool(name="bsp", bufs=1))
            gp = ctx.enter_context(tc.tile_pool(name="gates", bufs=1))
            ew = ctx.enter_context(tc.tile_pool(name="ew", bufs=1))
            wp = ctx.enter_context(tc.tile_pool(name="warm", bufs=1))

            # small PE warm-up burst overlapping the DMA prologue
            wa = wp.tile([P, P], FIN, tag="wa")
            wb = wp.tile([P, 512], FIN, tag="wb")
            nc.vector.memset(wa[:], 0.0)
            nc.vector.memset(wb[:], 0.0)
            eye = wp.tile([P, P], FIN, tag="eye")
            nc.sync.dma_start(eye[:], eye_d[:])

            # ---- mm1-x: atx[j][m-tile, 4*128] = sum_kc x[kc, j]^T-style ----
            # kc-outer over all 8 PSUM banks; x/LTx stream, 2 k-tiles per DMA.
            ptsx = [psp.tile([P, 4 * P], FP, tag="bank", name=f"ptx{j}")
                    for j in range(KT)]
            # PE warm-up burst into ptsx[0]: runs during the DMA prologue and
            # its garbage is discarded by the first real matmul's start=True
            # (keeps the PSUM pool at exactly 8 live allocations)
            for w in range(4):
                nc.tensor.matmul(ptsx[0][:], wa[:], wb[:], start=True,
                                 stop=True, skip_group_check=True)
            for kc2 in range(KT // 2):
                if kc2 == 0:
                    # quick-start: the first k-tile ships alone so mm1-x's
                    # first matmuls wait ~half as long for data
                    tt = tin.tile([P, 2 * M], FIN, tag="t")
                    lt = ltp.tile([P, 8 * P], FIN, tag="ltx")
                    nc.sync.dma_start(tt[:, 0:M], x_d[ts(0, P), 0:M])
                    nc.sync.dma_start(lt[:, 0:4 * P], ltx_d[ts(0, P), 0:4 * P])
                    nc.sync.dma_start(tt[:, M:2 * M], x_d[ts(0, P), M:2 * M])
                    nc.sync.dma_start(lt[:, 4 * P:8 * P],
                                      ltx_d[ts(0, P), 4 * P:8 * P])
                else:
                    tt = tin.tile([P, 2 * M], FIN, tag="t")
                    nc.sync.dma_start(tt[:], x_d[ts(kc2, P), :])
                    lt = ltp.tile([P, 8 * P], FIN, tag="ltx")
                    nc.sync.dma_start(lt[:], ltx_d[ts(kc2, P), :])
                for b in range(2):
                    kc = 2 * kc2 + b
                    for j in range(KT):
                        nc.tensor.matmul(
                            ptsx[j][:],
                            tt[:, b * M + j * P:b * M + (j + 1) * P],
                            lt[:, b * 4 * P:(b + 1) * 4 * P],
                            start=(kc == 0), stop=(kc == KT - 1))
            atx = []
            for j in range(KT):
                at = atxp.tile([P, 4 * P], FIN, tag=f"atx{j}")
                nc.vector.tensor_copy(at[:], ptsx[j][:])
                atx.append(at)

            # resident h k-tiles (2 per DMA) + per-group LTh (4 k-tiles/DMA),
            # queued after mm1-x DMAs; consumed from mm1-h onward
            hres2 = []
            for kc2 in range(KT // 2):
                ht = hrp.tile([P, 2 * M], FIN, tag=f"h{kc2}")
                nc.sync.dma_start(ht[:], h_d[ts(kc2, P), :])
                hres2.append(ht)

            def hres(kc):
                return hres2[kc // 2][:, (kc % 2) * M:(kc % 2 + 1) * M]

            lth2 = {}
            for gi_, grp in enumerate(("if", "go")):
                for half in range(2):
                    lh = lhp.tile([P, 8 * P], FIN, tag=f"lth{grp}{half}")
                    nc.sync.dma_start(lh[:], lth_d[ts(gi_, P),
                                                   ts(half, 8 * P)])
                    lth2[(grp, half)] = lh

            def lth(grp, kc):
                return lth2[(grp, kc // 4)][:, (kc % 4) * 2 * P:(kc % 4 + 1) * 2 * P]

            # all 4 pair biases in one DMA; c load (used near the end)
            bst = bsp.tile([P, 4 * M], FIN, tag="bs")
            nc.sync.dma_start(bst[:], bs_d[:])
            cs = ew.tile([P, M], FIN, tag="cs")
            nc.sync.dma_start(cs[:], c_d[:])

            def mm1_h(grp):
                """A^T for the two h-gates of `grp`: 8 tiles [128, 256].

                kc-outer so the first matmuls only need h[0] (PE can flow
                straight from the previous phase instead of waiting for the
                whole resident-h load).
                """
                ats = []
                for jg in (0, 4):  # two 4-bank j-groups (PSUM budget)
                    pts = [psp.tile([P, 2 * P], FP, tag="bank",
                                    name=f"pth{grp}{j}")
                           for j in range(jg, jg + 4)]
                    for kc in range(KT):
                        for ji, j in enumerate(range(jg, jg + 4)):
                            nc.tensor.matmul(pts[ji][:],
                                             hres(kc)[:, ts(j, P)],
                                             lth(grp, kc),
                                             start=(kc == 0),
                                             stop=(kc == KT - 1))
                    for ji, j in enumerate(range(jg, jg + 4)):
                        at = athp.tile([P, 2 * P], FIN, tag=f"ath{j}")
                        nc.vector.tensor_copy(at[:], pts[ji][:])
                        ats.append(at)
                return ats

            gates = []

            # scratch PSUM bank for same-weight filler matmuls (see below)
            fill_ps = psp.tile([P, 512], FP, tag="bank", name="fill_ps")

            def pair_stream(p, ath, hslot, tail_split=False):
                """R-stream + matmuls for gate pair p (accumulate in PSUM).

                Bias is folded in as an identity matmul at the start of the
                accumulation, so no vector-engine work is left between the
                last matmul and the activation.  R comes 4 k-tiles per DMA.

                For the late pairs (g, o) a third matmul per k-tile re-uses
                the SAME stationary weights into a scratch bank: pure PE-duty
                padding that keeps the HAM clock at 2.4 GHz through the
                DMA-paced end of the stream without extra LDWEIGHTS."""
                pt0 = psp.tile([P, 512], FP, tag="bank", name=f"p{p}b0")
                pt1 = psp.tile([P, 512], FP, tag="bank", name=f"p{p}b1")
                nc.tensor.matmul(pt0[:], eye[:], bst[:, p * M:p * M + 512],
                                 start=True, stop=False)
                nc.tensor.matmul(pt1[:], eye[:], bst[:, p * M + 512:(p + 1) * M],
                                 start=True, stop=False)
                for s, rd in (("x", rx_d[p]), ("h", rh_d[p])):
                    for j4 in range(2):
                        rt = rp.tile([P, 4 * M], FIN, tag="r")
                        nc.sync.dma_start(rt[:], rd[ts(j4, P), :])
                        if tail_split and s == "h" and j4 == 1:
                            # final transfer: run all of pt0's matmuls first
                            # so half 0's epilogue (act, h_new, store)
                            # overlaps half 1's matmuls
                            for half, pt in ((0, pt0), (1, pt1)):
                                for b in range(4):
                                    j = 4 + b
                                    lhsT = ath[j][:, ts(hslot, P)]
                                    nc.tensor.matmul(
                                        pt[:], lhsT,
                                        rt[:, b * M + half * 512:
                                            b * M + half * 512 + 512],
                                        start=False, stop=(b == 3))
                            continue
                        for b in range(4):
                            j = 4 * j4 + b
                            last = (s == "h") and (j == KT - 1)
                            if s == "x":
                                lhsT = atx[j][:, ts(p, P)]
                            else:
                                lhsT = ath[j][:, ts(hslot, P)]
                            nc.tensor.matmul(pt0[:], lhsT,
                                             rt[:, b * M:b * M + 512],
                                             start=False, stop=last)
                            nc.tensor.matmul(pt1[:], lhsT,
                                             rt[:, b * M + 512:(b + 1) * M],
                                             start=False, stop=last)
                            if p >= 2:
                                nc.tensor.matmul(fill_ps[:, 0:256], lhsT,
                                                 rt[:, b * M:b * M + 256],
                                                 start=True, stop=True,
                                                 skip_group_check=True)
                return pt0, pt1

            def pair_epilogue(p, pt0, pt1, actname):
                gt = gp.tile([P, M], FP, tag=f"g{p}")
                af = getattr(AF, actname)
                nc.scalar.activation(gt[:, 0:512], pt0[:], af)
                nc.scalar.activation(gt[:, 512:1024], pt1[:], af)
                gates.append(gt)

            def pair_mm(p, ath, hslot, actname):
                pt0, pt1 = pair_stream(p, ath, hslot)
                pair_epilogue(p, pt0, pt1, actname)

            # block {i, f}
            ath_if = mm1_h("if")
            pair_mm(0, ath_if, 0, PAIRS[0][2])
            pair_mm(1, ath_if, 1, PAIRS[1][2])
            # block {g, o}
            ath_go = mm1_h("go")
            pair_mm(2, ath_go, 0, PAIRS[2][2])
            gi, gf, gg = gates

            # o-gate matmul stream first: its R DMAs must queue BEFORE the
            # c-chain's cn stores on the sync queue (FIFO head-blocking).
            pt0_o, pt1_o = pair_stream(3, ath_go, 1, tail_split=True)

            # c_new chain: executes during the o-gate matmuls (deps on g/i/f)
            fc = ew.tile([P, M], FP, tag="fc")
            ig = ew.tile([P, M], FP, tag="ig")
            cn = ew.tile([P, M], FIN, tag="cn")
            tch = ew.tile([P, M], FP, tag="tch")
            for hf in range(2):
                sl = ts(hf, 512)
                nc.vector.tensor_mul(fc[:, sl], gf[:, sl], cs[:, sl])
                nc.vector.tensor_mul(ig[:, sl], gi[:, sl], gg[:, sl])
                nc.vector.tensor_add(cn[:, sl], fc[:, sl], ig[:, sl])
                nc.sync.dma_start(cn_d[:, sl], cn[:, sl])
                nc.scalar.activation(tch[:, sl], cn[:, sl], AF.Tanh)

            pair_epilogue(3, pt0_o, pt1_o, PAIRS[3][2])  # o
            go = gates[3]
            hn = ew.tile([P, M], FIN, tag="hn")
            for hf in range(2):
                sl = ts(hf, 512)
                nc.vector.tensor_mul(hn[:, sl], go[:, sl], tch[:, sl])
                nc.sync.dma_start(hn_d[:, sl], hn[:, sl])

    nc.compile()
    return nc


def _get_program():
    if "nc" not in _cache:
        _cache["nc"] = _build_program()
    return _cache["nc"]


def _pack_ktiles(a, group):
    """[K, W] -> [K//group, group*W]: `group` consecutive 128-row k-tiles
    side by side in the free dim (so one DMA moves them all)."""
    K_, W = a.shape
    nt = K_ // P
    return np.ascontiguousarray(
        a.reshape(nt // group, group, P, W).transpose(0, 2, 1, 3)
        .reshape(K_ // group, group * W))


def _prep_in_maps(inputs):
    from ml_dtypes import bfloat16

    bf = lambda a: np.ascontiguousarray(np.asarray(a, dtype=np.float32).astype(bfloat16))
    x = bf(inputs["x"]); h = bf(inputs["h"]); c = bf(inputs["c"])
    eye = np.eye(P, dtype=np.float32).astype(bfloat16)
    LTx = [bf(np.asarray(inputs[f"L_{xg}"]).T) for xg, _, _ in PAIRS]
    LTh = [bf(np.asarray(inputs[f"L_{hg}"]).T) for _, hg, _ in PAIRS]
    Rx = [_pack_ktiles(bf(inputs[f"R_{xg}"]), 4) for xg, _, _ in PAIRS]
    Rh = [_pack_ktiles(bf(inputs[f"R_{hg}"]), 4) for _, hg, _ in PAIRS]
    bsum = [bf(np.asarray(inputs[f"b_{xg}"]) + np.asarray(inputs[f"b_{hg}"]))
            for xg, hg, _ in PAIRS]
    xp = _pack_ktiles(x, 2)
    hp = _pack_ktiles(h, 2)

    in_maps = []
    for k in range(NC):
        sl = slice(P * k, P * (k + 1))
        ltx = np.concatenate([lt[:, sl] for lt in LTx], axis=1)  # [N, 512]
        lth = np.concatenate([lt[:, sl] for lt in LTh], axis=1)  # [N, 512]
        # lth groups: "if" = cols 0:256, "go" = cols 256:512; each packed as
        # [128, 2048] = (row half) x (4 k-tiles of [128,256] side by side),
        # then stacked into [256, 2048]
        lth_rows = []
        for g0 in (0, 1):
            grp = lth[:, 256 * g0:256 * (g0 + 1)]          # [1024, 256]
            packed = _pack_ktiles(grp, 4)                  # [256, 1024]
            lth_rows.append(packed.reshape(2, P, 4 * 256)
                            .transpose(1, 0, 2).reshape(P, 8 * 256))
        im = {
            "x": xp, "h": hp,
            "ltx": _pack_ktiles(ltx, 2),
            "lth": np.ascontiguousarray(np.concatenate(lth_rows, axis=0)),
            "bsum": np.ascontiguousarray(
                np.concatenate([b[sl] for b in bsum], axis=1)),
            "cprev": np.ascontiguousarray(c[sl]),
            "eye": eye,
        }
        for p in range(4):
            im[f"rx{p}"] = Rx[p]
            im[f"rh{p}"] = Rh[p]
        in_maps.append(im)
    return in_maps


def kernel(**inputs):
    from concourse.bass_utils import run_bass_kernel_spmd

    nc = _get_program()
    in_maps = _prep_in_maps(inputs)
    res = run_bass_kernel_spmd(nc, in_maps, core_ids=list(range(NC)))
    h_new = np.concatenate(
        [np.asarray(res.results[k]["h_new"], dtype=np.float32) for k in range(NC)],
        axis=0)
    c_new = np.concatenate(
        [np.asarray(res.results[k]["c_new"], dtype=np.float32) for k in range(NC)],
        axis=0)
    return (h_new, c_new)
